# revision 1
# baseline (speedup 1.0000x reference)
"""2-layer GAT (PyG GATConv, concat=False, self-loops) on 8 Trainium2 cores.

Strategy (sharding_hint): nodes/edges partitioned by destination across 8
cores; each core owns a 6250-dst range so scatter-softmax and segment-sum
are core-local. Within a core, dsts are degree-sorted and grouped into
49 blocks of 128 lanes (padded CSR): each PSUM partition lane owns one dst,
edge slots are accumulated with an identity-weight matmul. Per-edge source
rows (asrc + h, bf16 features) are fetched with dma_gather from a node
table in HBM; the table is split lo/hi around a sentinel row to fit int16
indices. The softmax never needs a segment-max (exp arguments are O(1));
normalization happens per-dst after aggregation. Layer boundary is an
AllGather collective of the (transposed, ELU'd) layer-1 output.
"""
import sys
sys.path.insert(0, "/opt/trn_rl_repo")

import numpy as np
import ml_dtypes

import concourse.bass as bass
import concourse.bacc as bacc
import concourse.mybir as mybir
from concourse.bass_utils import run_bass_kernel_spmd
from concourse.tile import TileContext

N = 50000
E = 1600000
IN = 128
H = 4
F = 32
NEG = 0.2
NCORES = 8
PERC = N // NCORES          # 6250
NBLK = (PERC + 127) // 128  # 49
SENT = N // 2               # sentinel row index in the node table (25000)
RW = 128                    # table row: 128 f32 slots = 512B
TROWS = N + 1


def _pack_idx(idx_flat):
    """[n] -> [128, n/16] int16; idx i -> (partition i%16, col i//16), x8 replicated."""
    n = idx_flat.shape[0]
    assert n % 16 == 0
    a = idx_flat.reshape(n // 16, 16).T.astype(np.int16)
    return np.ascontiguousarray(np.tile(a, (8, 1)))


def _interleave_w(w):
    """[..., H*F] (h-major) -> [..., F*H] (f-major, h-minor) column order."""
    return np.ascontiguousarray(
        w.reshape(*w.shape[:-1], H, F).swapaxes(-1, -2).reshape(*w.shape[:-1], H * F))


def _row_of(n):
    """node/g id -> table row (sentinel at SENT)."""
    return np.where(n < SENT, n, n + 1)


def _preprocess(edge_index):
    """All index preprocessing. Returns per-core tensors + shared structure."""
    src0 = np.concatenate([edge_index[0], np.arange(N, dtype=np.int64)])
    dst0 = np.concatenate([edge_index[1], np.arange(N, dtype=np.int64)])
    core_of = dst0 // PERC

    # per-core degree-sorted permutation of its dst range
    perms = []          # perms[c][pos] = absolute dst id
    g_of = np.empty(N, np.int64)   # node -> global permuted position
    deg = np.bincount(dst0, minlength=N)
    for c in range(NCORES):
        ids = np.arange(c * PERC, (c + 1) * PERC)
        order = np.argsort(-deg[ids], kind="stable")
        perm = ids[order]
        perms.append(perm)
        g_of[perm] = c * PERC + np.arange(PERC)

    # bucket edges per (core, lane-position), split lo/hi by src
    # adj[c][pos] = (lo_src_list, hi_src_list)
    pos_of_dst = g_of % PERC  # position within its core
    lane_lo = [[[] for _ in range(PERC)] for _ in range(NCORES)]
    lane_hi = [[[] for _ in range(PERC)] for _ in range(NCORES)]
    order = np.lexsort((src0, dst0))
    s_sorted, d_sorted = src0[order], dst0[order]
    c_sorted = d_sorted // PERC
    p_sorted = pos_of_dst[d_sorted]
    lo_mask = s_sorted < SENT
    for c in range(NCORES):
        m = c_sorted == c
        for p, s, lo in zip(p_sorted[m], s_sorted[m], lo_mask[m]):
            (lane_lo if lo else lane_hi)[c][p].append(s)

    # per-block slot counts, max across cores
    n_lo = np.zeros(NBLK, np.int64)
    n_hi = np.zeros(NBLK, np.int64)
    for c in range(NCORES):
        for b in range(NBLK):
            lanes = range(b * 128, min((b + 1) * 128, PERC))
            n_lo[b] = max(n_lo[b], max(len(lane_lo[c][p]) for p in lanes))
            n_hi[b] = max(n_hi[b], max(len(lane_hi[c][p]) for p in lanes))

    # idx streams per core per layer: for each block: n_lo lo-slots then n_hi hi-slots
    # layer1 idx: table row of absolute src; layer2: row of g_of[src]
    def build_gidx(c, use_g):
        cols = []
        for b in range(NBLK):
            lanes = [b * 128 + i for i in range(128)]
            for k in range(int(n_lo[b])):
                col = np.full(128, SENT, np.int64)  # lo sentinel
                for i, p in enumerate(lanes):
                    if p < PERC and k < len(lane_lo[c][p]):
                        s = lane_lo[c][p][k]
                        v = g_of[s] if use_g else s
                        col[i] = v  # v < SENT: row = v
                cols.append(col)
            for k in range(int(n_hi[b])):
                col = np.zeros(128, np.int64)       # hi sentinel (row SENT -> idx 0)
                for i, p in enumerate(lanes):
                    if p < PERC and k < len(lane_hi[c][p]):
                        s = lane_hi[c][p][k]
                        v = g_of[s] if use_g else s
                        col[i] = v + 1 - SENT       # hi view row index
                cols.append(col)
        return np.concatenate([_pack_idx(col) for col in cols], axis=1)

    # NOTE: lo/hi membership differs between absolute and g numbering!
    # A src s belongs to lo iff (its table index) < SENT.  For layer 2 the
    # table is in g space, so the lo/hi split must use g_of[s].  Since
    # g_of[s] // PERC == s // PERC (permutation is within-core), s < SENT
    # <=> g_of[s] < SENT.  (SENT = 4*PERC boundary is core-aligned.)
    assert SENT % PERC == 0

    d_lo, d_hi, d_lo2, d_hi2 = [], [], [], []
    for c in range(NCORES):
        dl = np.full(NBLK * 128, SENT, np.int64)
        dh = np.zeros(NBLK * 128, np.int64)
        dl2 = np.full(NBLK * 128, SENT, np.int64)
        dh2 = np.zeros(NBLK * 128, np.int64)
        for pos in range(PERC):
            dd = perms[c][pos]
            g = c * PERC + pos
            if dd < SENT:
                dl[pos] = dd
            else:
                dh[pos] = dd + 1 - SENT
            if g < SENT:
                dl2[pos] = g
            else:
                dh2[pos] = g + 1 - SENT
        d_lo.append(_pack_idx(dl)); d_hi.append(_pack_idx(dh))
        d_lo2.append(_pack_idx(dl2)); d_hi2.append(_pack_idx(dh2))

    gidx1 = [build_gidx(c, False) for c in range(NCORES)]
    gidx2 = [build_gidx(c, True) for c in range(NCORES)]
    return dict(gidx1=gidx1, gidx2=gidx2, d_lo=d_lo, d_hi=d_hi,
                d_lo2=d_lo2, d_hi2=d_hi2, n_lo=n_lo, n_hi=n_hi,
                perms=perms, g_of=g_of)


def _stage_a(nc, tc, pools, src_dram, slabs, waug_sb, tbl, kdim):
    """h = src @ Waug -> table rows [asrc|adst|h_bf16|0pad] written to tbl.

    slabs: list of (view, widths, row0, split) — `view` is a [kdim, W] lhsT
    column slab (W = sum(widths) <= 16*128), processed as len(widths) matmul
    tiles; the packed rows go out as ONE slab DMA to tbl[row0:row0+W] (rows
    shifted +1 above the sentinel split if `split` is not None).
    """
    sb, ps = pools
    for (view, widths, row0, split) in slabs:
        W = sum(widths)
        nt = len(widths)
        xsb = sb.tile([kdim, 16 * 128], mybir.dt.float32, tag="xa")
        nc.sync.dma_start(out=xsb[:, 0:W], in_=view)
        stg = sb.tile([128, 16, RW], mybir.dt.float32, tag="sa")
        c0 = 0
        for t in range(nt):
            w = widths[t]
            psum = ps.tile([128, 136], mybir.dt.float32, tag="pa")
            nc.tensor.matmul(out=psum[0:w, :], lhsT=xsb[:, c0:c0 + w],
                             rhs=waug_sb[:], start=True, stop=True)
            nc.scalar.activation(out=stg[0:w, t, 0:8], in_=psum[0:w, 128:136],
                                 func=mybir.ActivationFunctionType.Copy)
            nc.vector.tensor_copy(
                out=stg[0:w, t, 8:72].bitcast(mybir.dt.bfloat16),
                in_=psum[0:w, 0:128])
            nc.vector.memset(stg[0:w, t, 72:RW], 0.0)
            c0 += w
        # slab write: row (t*128+p) -> tbl[row0 + t*128 + p]
        assert all(wd == 128 for wd in widths[:-1])
        wlast = widths[-1]
        out_view = tbl[row0:row0 + W, :].rearrange("(t p) c -> p t c", p=128) \
            if wlast == 128 else None
        if split is not None:
            # slab straddles the sentinel row: per-tile writes around it
            for t in range(nt):
                w = widths[t]
                off = t * 128              # offset of tile within slab
                r = row0 + off
                if off + w <= split:
                    parts = [(0, w, r)]
                elif off >= split:
                    parts = [(0, w, r + 1)]
                else:
                    k = split - off
                    parts = [(0, k, r), (k, w - k, r + k + 1)]
                for (o, ww, rdst) in parts:
                    nc.sync.dma_start(out=tbl[rdst:rdst + ww, :],
                                      in_=stg[o:o + ww, t, :])
        elif wlast == 128:
            nc.sync.dma_start(out=out_view, in_=stg[:, 0:nt, :])
        else:
            if nt > 1:
                nc.sync.dma_start(
                    out=tbl[row0:row0 + (nt - 1) * 128, :].rearrange(
                        "(t p) c -> p t c", p=128),
                    in_=stg[:, 0:nt - 1, :])
            nc.sync.dma_start(out=tbl[row0 + (nt - 1) * 128:row0 + W, :],
                              in_=stg[0:wlast, nt - 1, :])


def _build_program(n_lo, n_hi, upto=99):
    S = int((n_lo + n_hi).sum())          # total slots per layer
    nc = bacc.Bacc("TRN2", target_bir_lowering=False, debug=False,
                   num_devices=NCORES)

    f32, bf16, i16 = mybir.dt.float32, mybir.dt.bfloat16, mybir.dt.int16
    xT = nc.dram_tensor("xT", [128, N], f32, kind="ExternalInput")
    waug1 = nc.dram_tensor("waug1", [IN, 136], f32, kind="ExternalInput")
    waug2 = nc.dram_tensor("waug2", [F, 136], f32, kind="ExternalInput")
    gidx1 = nc.dram_tensor("gidx1", [128, S * 8], i16, kind="ExternalInput")
    gidx2 = nc.dram_tensor("gidx2", [128, S * 8], i16, kind="ExternalInput")
    dlo1 = nc.dram_tensor("dlo1", [128, NBLK * 8], i16, kind="ExternalInput")
    dhi1 = nc.dram_tensor("dhi1", [128, NBLK * 8], i16, kind="ExternalInput")
    dlo2 = nc.dram_tensor("dlo2", [128, NBLK * 8], i16, kind="ExternalInput")
    dhi2 = nc.dram_tensor("dhi2", [128, NBLK * 8], i16, kind="ExternalInput")
    identb = nc.dram_tensor("identb", [128, 128], bf16, kind="ExternalInput")
    identf = nc.dram_tensor("identf", [128, 128], f32, kind="ExternalInput")
    sentrow = nc.dram_tensor("sentrow", [1, RW], f32, kind="ExternalInput")
    b1r = nc.dram_tensor("b1r", [128, F], f32, kind="ExternalInput")
    b2r = nc.dram_tensor("b2r", [128, F], f32, kind="ExternalInput")

    T1 = nc.dram_tensor("T1", [TROWS, RW], f32)
    T2 = nc.dram_tensor("T2", [TROWS, RW], f32)
    o1T = nc.dram_tensor("o1T", [F, PERC], f32)
    o1Tg = nc.dram_tensor("o1Tg", [NCORES * F, PERC], f32, addr_space="Shared")
    out2p = nc.dram_tensor("out2p", [PERC, F], f32, kind="ExternalOutput")

    T1_lo, T1_hi = T1[0:SENT + 1, :], T1[SENT:TROWS, :]
    T2_lo, T2_hi = T2[0:SENT + 1, :], T2[SENT:TROWS, :]

    with TileContext(nc) as tc:
        with (
            tc.tile_pool(name="cons", bufs=1) as cons,
            tc.tile_pool(name="sbA", bufs=3) as sbA,
            tc.tile_pool(name="psA", bufs=4, space="PSUM") as psA,
            tc.tile_pool(name="dgp", bufs=2) as dgp,
            tc.tile_pool(name="gp", bufs=6) as gp,
            tc.tile_pool(name="rp", bufs=6) as rp,
            tc.tile_pool(name="ep", bufs=6) as ep,
            tc.tile_pool(name="psE", bufs=3, space="PSUM") as psE,
            tc.tile_pool(name="psT", bufs=1, space="PSUM") as psT,
        ):
            identb_sb = cons.tile([128, 128], bf16)
            nc.sync.dma_start(out=identb_sb[:], in_=identb[:, :])
            identf_sb = cons.tile([128, 128], f32)
            nc.sync.dma_start(out=identf_sb[:], in_=identf[:, :])
            waug1_sb = cons.tile([IN, 136], f32)
            nc.sync.dma_start(out=waug1_sb[:], in_=waug1[:, :])
            waug2_sb = cons.tile([F, 136], f32)
            nc.sync.dma_start(out=waug2_sb[:], in_=waug2[:, :])
            b1r_sb = cons.tile([128, F], f32)
            nc.sync.dma_start(out=b1r_sb[:], in_=b1r[:, :])
            b2r_sb = cons.tile([128, F], f32)
            nc.sync.dma_start(out=b2r_sb[:], in_=b2r[:, :])
            sent_sb = cons.tile([1, RW], f32)
            nc.sync.dma_start(out=sent_sb[:], in_=sentrow[:, :])
            nc.sync.dma_start(out=T1[SENT:SENT + 1, :], in_=sent_sb[:])
            nc.sync.dma_start(out=T2[SENT:SENT + 1, :], in_=sent_sb[:])

            import os as _os
            SUB = int(_os.environ.get("GAT_SUB", "99"))

            def edge_layer(tbl_lo, tbl_hi, gidx_sb, dlo_sb, dhi_sb, bias_sb,
                           is_layer1):
                # D-gather: adst of the block lanes (lo+hi sentinel-add),
                # in pieces of <=7 blocks (dma_gather ring limit ~1008 idxs)
                dsb = cons.tile([128, NBLK, 4], f32,
                                tag="dsb1" if is_layer1 else "dsb2")
                for half, (view, ib) in enumerate([(tbl_lo, dlo_sb),
                                                   (tbl_hi, dhi_sb)]):
                    for p0 in range(0, NBLK, 7):
                        pn = min(7, NBLK - p0)
                        dg = dgp.tile([128, 7, RW], f32, tag="dg")
                        nc.gpsimd.dma_gather(
                            dg[:, 0:pn, :], view, ib[:, p0 * 8:(p0 + pn) * 8],
                            pn * 128, pn * 128, RW)
                        if half == 0:
                            nc.vector.tensor_copy(out=dsb[:, p0:p0 + pn, :],
                                                  in_=dg[:, 0:pn, 4:8])
                        else:
                            nc.vector.tensor_tensor(
                                out=dsb[:, p0:p0 + pn, :],
                                in0=dsb[:, p0:p0 + pn, :],
                                in1=dg[:, 0:pn, 4:8],
                                op=mybir.AluOpType.add)

                if SUB == 0:
                    return
                col = 0
                for b in range(NBLK):
                    w_b = min(128, PERC - b * 128)
                    psum_b = psE.tile([128, 132], f32, tag="acc")
                    first = True
                    tot_b = int(n_lo[b]) + int(n_hi[b])
                    done = 0
                    for half in range(2):
                        nsl_all = int(n_lo[b] if half == 0 else n_hi[b])
                        view = tbl_lo if half == 0 else tbl_hi
                        for s0 in range(0, nsl_all, 7):
                            nsl = min(7, nsl_all - s0)
                            g_sb = gp.tile([128, 7, RW], f32, tag="g")
                            nc.gpsimd.dma_gather(
                                g_sb[:, 0:nsl, :], view,
                                gidx_sb[:, col:col + nsl * 8],
                                nsl * 128, nsl * 128, RW)
                            col += nsl * 8
                            if SUB == 1:
                                continue
                            e_sb = ep.tile([128, 7, 4], f32, tag="e")
                            nc.vector.tensor_tensor(
                                out=e_sb[:, 0:nsl, :], in0=g_sb[:, 0:nsl, 0:4],
                                in1=dsb[:, b, :].unsqueeze(1).to_broadcast(
                                    [128, nsl, 4]),
                                op=mybir.AluOpType.add)
                            e2_sb = ep.tile([128, 7, 4], f32, tag="e2")
                            nc.vector.tensor_scalar(
                                out=e2_sb[:, 0:nsl, :], in0=e_sb[:, 0:nsl, :],
                                scalar1=NEG, scalar2=None,
                                op0=mybir.AluOpType.mult)
                            nc.vector.tensor_tensor(
                                out=e2_sb[:, 0:nsl, :], in0=e2_sb[:, 0:nsl, :],
                                in1=e_sb[:, 0:nsl, :], op=mybir.AluOpType.max)
                            # clamp: keep the ACT exp LUT in-domain
                            nc.vector.tensor_scalar(
                                out=e2_sb[:, 0:nsl, :], in0=e2_sb[:, 0:nsl, :],
                                scalar1=-80.0, scalar2=None,
                                op0=mybir.AluOpType.max)
                            if SUB == 2:
                                continue
                            rhs_sb = rp.tile([128, 7, 132], bf16, tag="rhs")
                            nc.scalar.activation(
                                out=rhs_sb[:, 0:nsl, 0:4], in_=e2_sb[:, 0:nsl, :],
                                func=mybir.ActivationFunctionType.Exp)
                            nc.vector.tensor_tensor(
                                out=rhs_sb[:, 0:nsl, 4:132].rearrange(
                                    "p n (f h) -> p n f h", h=H),
                                in0=g_sb[:, 0:nsl, 8:72].bitcast(bf16).rearrange(
                                    "p n (f h) -> p n f h", h=H),
                                in1=rhs_sb[:, 0:nsl, 0:4].unsqueeze(2).to_broadcast(
                                    [128, nsl, F, H]),
                                op=mybir.AluOpType.mult)
                            if SUB == 3:
                                continue
                            for k in range(nsl):
                                done += 1
                                nc.tensor.matmul(
                                    out=psum_b[:], lhsT=identb_sb[:],
                                    rhs=rhs_sb[:, k, :],
                                    start=first,
                                    stop=(done == tot_b))
                                first = False

                    if SUB <= 4:
                        continue
                    # epilogue: out = sum_h U[:, (f,h)] / (s_h+eps) / H (+bias)
                    sden = ep.tile([128, 4], f32, tag="sden")
                    nc.vector.tensor_scalar(
                        out=sden[:], in0=psum_b[:, 0:4], scalar1=1e-16,
                        scalar2=None, op0=mybir.AluOpType.add)
                    rv = ep.tile([128, 4], f32, tag="rv")
                    nc.vector.reciprocal(out=rv[:], in_=sden[:])
                    mt = ep.tile([128, 128], f32, tag="mt")
                    nc.vector.tensor_tensor(
                        out=mt[:].rearrange("p (f h) -> p f h", h=H),
                        in0=psum_b[:, 4:132].rearrange("p (f h) -> p f h", h=H),
                        in1=rv[:].unsqueeze(1).to_broadcast([128, F, H]),
                        op=mybir.AluOpType.mult)
                    of = ep.tile([128, F], f32, tag="of")
                    nc.vector.tensor_reduce(
                        out=of[:], in_=mt[:].rearrange("p (f h) -> p f h", h=H),
                        axis=mybir.AxisListType.X, op=mybir.AluOpType.add)
                    ob = ep.tile([128, F], f32, tag="ob")
                    nc.vector.tensor_scalar(
                        out=ob[:], in0=of[:], scalar1=1.0 / H, scalar2=None,
                        op0=mybir.AluOpType.mult)
                    nc.vector.tensor_tensor(
                        out=ob[:], in0=ob[:], in1=bias_sb[:],
                        op=mybir.AluOpType.add)
                    if is_layer1:
                        # ELU then transpose to o1T[:, block cols]
                        m0 = ep.tile([128, F], f32, tag="m0")
                        nc.vector.tensor_scalar(
                            out=m0[:], in0=ob[:], scalar1=0.0, scalar2=None,
                            op0=mybir.AluOpType.min)
                        em = ep.tile([128, F], f32, tag="em")
                        nc.scalar.activation(
                            out=em[:], in_=m0[:],
                            func=mybir.ActivationFunctionType.Exp)
                        nc.vector.tensor_scalar(
                            out=em[:], in0=em[:], scalar1=-1.0, scalar2=None,
                            op0=mybir.AluOpType.add)
                        nc.vector.tensor_tensor(
                            out=ob[:], in0=ob[:], in1=em[:],
                            op=mybir.AluOpType.max)
                        pT = psT.tile([F, 128], f32, tag="pT")
                        nc.tensor.transpose(out=pT[:], in_=ob[:],
                                            identity=identf_sb[:])
                        oT = ep.tile([F, 128], f32, tag="oT")
                        nc.scalar.activation(
                            out=oT[:], in_=pT[:],
                            func=mybir.ActivationFunctionType.Copy)
                        nc.sync.dma_start(
                            out=o1T[:, b * 128:b * 128 + w_b],
                            in_=oT[:, 0:w_b])
                    else:
                        nc.sync.dma_start(
                            out=out2p[b * 128:b * 128 + w_b, :],
                            in_=ob[0:w_b, :])

            # ---- layer 1 ----
            gidx1_sb = cons.tile([128, S * 8], i16, tag="gidx")
            nc.sync.dma_start(out=gidx1_sb[:], in_=gidx1[:, :])
            dlo1_sb = cons.tile([128, NBLK * 8], i16, tag="dlo1")
            nc.sync.dma_start(out=dlo1_sb[:], in_=dlo1[:, :])
            dhi1_sb = cons.tile([128, NBLK * 8], i16, tag="dhi1")
            nc.sync.dma_start(out=dhi1_sb[:], in_=dhi1[:, :])

            def dbg_dump(src_dram, rows, cols):
                # copy a DRAM region into out2p (debug only)
                for t in range((rows + 127) // 128):
                    r0 = t * 128
                    w = min(128, rows - r0)
                    tt = ep.tile([128, F], f32, tag="dbg")
                    nc.sync.dma_start(out=tt[0:w, 0:cols],
                                      in_=src_dram[r0:r0 + w, 0:cols])
                    nc.sync.dma_start(out=out2p[r0:r0 + w, :], in_=tt[0:w, :])

            views1 = []
            SL = 16 * 128
            for s0 in range(0, N, SL):
                Ws = min(SL, N - s0)
                widths = [128] * (Ws // 128) + ([Ws % 128] if Ws % 128 else [])
                row0 = s0 if s0 < SENT else s0 + 1
                split = SENT - s0 if (s0 < SENT < s0 + Ws) else None
                views1.append((xT[:, s0:s0 + Ws], widths, row0, split))
            _stage_a(nc, tc, (sbA, psA), xT, views1, waug1_sb, T1, IN)

            if upto == 1:
                dbg_dump(T1, PERC, 32)
            if upto >= 2:
                edge_layer(T1_lo, T1_hi, gidx1_sb, dlo1_sb, dhi1_sb, b1r_sb, True)
            if upto == 2:
                # flat copy o1T [F, PERC] -> out2p storage (host un-flattens)
                o2flat = out2p.ap().rearrange("a b -> (a b)")
                for f in range(F):
                    nc.sync.dma_start(out=o2flat[f * PERC:(f + 1) * PERC],
                                      in_=o1T.ap().rearrange("a b -> (a b)")[
                                          f * PERC:(f + 1) * PERC])

            # ---- allgather layer-1 output ----
            if upto >= 3:
                nc.gpsimd.collective_compute(
                    "AllGather", mybir.AluOpType.bypass,
                    replica_groups=[list(range(NCORES))],
                    ins=[o1T[:].opt()], outs=[o1Tg[:].opt()])
            if upto == 3:
                dbg_dump(o1Tg.ap().rearrange("a b -> b a"), PERC, 32)

            # ---- layer 2 ----
            if upto >= 4:
                gidx2_sb = cons.tile([128, S * 8], i16, tag="gidx")
                nc.sync.dma_start(out=gidx2_sb[:], in_=gidx2[:, :])
                dlo2_sb = cons.tile([128, NBLK * 8], i16, tag="dlo2")
                nc.sync.dma_start(out=dlo2_sb[:], in_=dlo2[:, :])
                dhi2_sb = cons.tile([128, NBLK * 8], i16, tag="dhi2")
                nc.sync.dma_start(out=dhi2_sb[:], in_=dhi2[:, :])

            views2 = []
            SL = 16 * 128
            for r in range(NCORES):
                for p0 in range(0, PERC, SL):
                    Ws = min(SL, PERC - p0)
                    widths = [128] * (Ws // 128) + ([Ws % 128] if Ws % 128 else [])
                    g0 = r * PERC + p0
                    row0 = g0 if g0 < SENT else g0 + 1
                    views2.append((o1Tg[r * F:(r + 1) * F, p0:p0 + Ws],
                                   widths, row0, None))
            if upto >= 4:
                _stage_a(nc, tc, (sbA, psA), o1Tg, views2, waug2_sb, T2, F)
            if upto == 4:
                dbg_dump(T2, PERC, 32)
            if upto >= 5:
                edge_layer(T2_lo, T2_hi, gidx2_sb, dlo2_sb, dhi2_sb, b2r_sb, False)

    nc.compile()
    return nc


_CACHE = {}


def _prepare(x, edge_index, W1, att_src1, att_dst1, b1, W2, att_src2,
             att_dst2, b2):
    x = np.asarray(x, np.float32)
    edge_index = np.asarray(edge_index, np.int64)
    key = hash(edge_index.tobytes())
    if key in _CACHE:
        meta, nc = _CACHE[key]
    else:
        meta = _preprocess(edge_index)
        nc = _build_program(meta["n_lo"], meta["n_hi"])
        _CACHE[key] = (meta, nc)

    W1 = np.asarray(W1, np.float32); W2 = np.asarray(W2, np.float32)
    a_s1 = np.asarray(att_src1, np.float32); a_d1 = np.asarray(att_dst1, np.float32)
    a_s2 = np.asarray(att_src2, np.float32); a_d2 = np.asarray(att_dst2, np.float32)
    b1 = np.asarray(b1, np.float32); b2 = np.asarray(b2, np.float32)

    def waug(W, a_s, a_d):
        # cols 0:128 interleaved W; 128:132 W@A_src; 132:136 W@A_dst
        ws = np.einsum("ihf,hf->ih", W.reshape(-1, H, F), a_s)
        wd = np.einsum("ihf,hf->ih", W.reshape(-1, H, F), a_d)
        return np.ascontiguousarray(
            np.concatenate([_interleave_w(W), ws, wd], axis=1).astype(np.float32))

    xT = np.ascontiguousarray(x.T)
    sentrow = np.zeros((1, RW), np.float32)
    sentrow[0, 0:4] = -1e30
    identb = np.eye(128, dtype=ml_dtypes.bfloat16)
    identf = np.eye(128, dtype=np.float32)
    b1r = np.broadcast_to(b1, (128, F)).copy()
    b2r = np.broadcast_to(b2, (128, F)).copy()

    common = dict(xT=xT, waug1=waug(W1, a_s1, a_d1), waug2=waug(W2, a_s2, a_d2),
                  identb=identb, identf=identf, sentrow=sentrow, b1r=b1r, b2r=b2r)
    in_maps = []
    for c in range(NCORES):
        in_maps.append(dict(common,
                            gidx1=meta["gidx1"][c], gidx2=meta["gidx2"][c],
                            dlo1=meta["d_lo"][c], dhi1=meta["d_hi"][c],
                            dlo2=meta["d_lo2"][c], dhi2=meta["d_hi2"][c]))
    return nc, in_maps, meta


def _assemble(meta, results):
    out = np.empty((N, F), np.float32)
    for c in range(NCORES):
        out[meta["perms"][c]] = results[c]["out2p"]
    return out


def kernel(**inputs):
    nc, in_maps, meta = _prepare(**inputs)
    res = run_bass_kernel_spmd(nc, in_maps, core_ids=list(range(NCORES)))
    return _assemble(meta, res.results)


def run_traced(**inputs):
    """Profiled run; returns BassKernelResults (exec_time_ns etc.)."""
    nc, in_maps, meta = _prepare(**inputs)
    res = run_bass_kernel_spmd(nc, in_maps, core_ids=list(range(NCORES)),
                               trace=True)
    res.gat_output = _assemble(meta, res.results)
    return res

